# revision 10
# baseline (speedup 1.0000x reference)
"""Trainium2 Bass kernel for nn_MoEsparseRoutingForClassification.

Reference computation (B=64, S=128, H=1024, E=8, L=2):
    x = X[:, 0, :]                                   # CLS token [B,H]
    y[b,o]   = sum_e g[b,e] * (x[b] . dense_w[e,o,:]) + (g @ dense_b)[b,o]
    t        = tanh(y)
    out[b,l] = sum_e g[b,e] * (t[b] . out_w[e,l,:])  + (g @ out_b)[b,l]

Distribution: the H output dim of the dense layer is sharded 8 ways
(OC=128 per core).  Core c computes y[:, c*OC:(c+1)*OC], applies tanh,
and contracts against out_w[:, :, c_slice] for a partial [B,L] logit;
partials sum on the host.  No cross-core collective.

v2 (fp8 stream): dense_w streams as float8 e3m4 (4 mantissa bits),
pre-scaled by Sw=128 so the N(0,0.02) weights land in e3m4's normal
range (max 15.5); the CLS block stays bf16 (its bytes ride the same
fp8 DRAM tensor via bitcast) and the PE runs mixed bf16(stationary)
x fp8(moving) matmuls.  HBM traffic drops to ~1.2 MiB/core.  The
Sw dequant is folded into existing ops: the dense_b K=1 ride-along
matmul uses a ones-row of value Sw (so PSUM holds Sw*(x.W + db))
and the tanh activation applies scale=1/Sw.  Host-measured rel err
~1.4e-2 (tolerance 2e-2); bf16 everywhere was 5.3e-3.

Scheduling (from NTFF traces of the bf16 predecessor @25.9us):
  - ~6.3us framework preamble before the first DMA dispatch and ~9us
    epilogue cascade after the last instruction are fixed costs, but
    the measured exec window appears to START after the preamble while
    INCLUDING the epilogue - so tail latency matters more than head.
  - weight stream all on the sync HW-DGE ring, 4 chunks the PE chases
    (xt+k0k1k2 | k3k4 | k5k6 | k7); ep+gc on the scalar ring which
    only gets descriptor service after sync's queued work - with the
    2x shorter fp8 stream the scalar-ring ep lands AFTER the PE's
    k1->k2 bubble, so the dense_b ride-along matmul moved to after k7
    (the PE queue is in-order; a parked not-ready matmul stalls it).
  - gate mix: tensor_tensor mult then a single strided tensor_reduce
    over e (fp32 acc out), replacing the 3-add pairwise tree.
  - output staging transposed on the DVE ([128,2] -> [2,128] via
    32x32 stream-transpose blocks) so the final DMA is 8 descriptors
    instead of 128: the teardown's wait on descriptor-completion
    pacing (~25ns each) was ~3us of the epilogue.
"""

import sys

import numpy as np

for _p in ("/opt/trn_rl_repo",):
    if _p not in sys.path:
        sys.path.insert(0, _p)

# If the environment sets BASS_TRACE but lacks antenv.axon_hooks (this agent
# image does), run_bass_kernel_spmd would crash on import; pre-seed a no-op
# module so tracing degrades gracefully instead.
try:  # pragma: no cover
    import antenv.axon_hooks  # noqa: F401
except Exception:  # pragma: no cover
    import types as _types

    _m = _types.ModuleType("antenv.axon_hooks")
    _m._hook = None
    _m.set_axon_ntff_profile_hook = lambda h: setattr(_m, "_hook", h)
    _m.get_axon_ntff_profile_hook = lambda: _m._hook
    sys.modules["antenv.axon_hooks"] = _m

from ml_dtypes import bfloat16, float8_e3m4  # noqa: E402

B, S, H = 64, 128, 1024
E, L = 8, 2
NCORES = 8
OC = H // NCORES          # dense-output slice per core (128)
HC = OC // 2              # half-slice mapped to a PSUM partition half (64)
KT = H // 128             # contraction tiles
P = 128

SW = 128.0                # fp8 pre-scale for dense_w (power of 2, exact)

XTC = KT * B              # xt cols (bf16 units) in the packed stream (512)
XT8 = 2 * XTC             # same bytes counted in fp8 cols (1024)
W1C = KT * 2 * E * HC     # w1 cols (8192, fp8)
XWC = XT8 + W1C           # packed stream width in fp8 cols (9216)

_cached = None


def _build():
    from contextlib import ExitStack

    import concourse.tile as tile
    from concourse import bacc, masks, mybir

    F32 = mybir.dt.float32
    BF16 = mybir.dt.bfloat16
    FP8 = mybir.dt.float8e3
    AF = mybir.ActivationFunctionType
    OP = mybir.AluOpType
    AX = mybir.AxisListType

    nc = bacc.Bacc("TRN2", target_bir_lowering=False, debug=False,
                   num_devices=NCORES)

    # E-pack along the free dim (one DMA for all E-partition consts):
    #   gt [E,B] | db [E,2,HC] | ow2 [E,2,L,HC] | ob [E,L] | gtz [E,P]
    EPACK = B + OC + L * OC + L + P      # 64+128+256+2+128 = 578
    # + a 1024-col dense-bias row block in PARTITION 0 ONLY (matmul
    # operands must sit at base partition 0/32/64): cols h*512+(e,hc) =
    # dense_b[e, h*64+hc], for the K=1 bias matmul
    EPACK2 = EPACK + 2 * E * HC          # 578+1024
    xw_d = nc.dram_tensor("xw", [P, XWC], FP8, kind="ExternalInput")
    ep_d = nc.dram_tensor("ep", [E, EPACK2], BF16, kind="ExternalInput")
    gc_d = nc.dram_tensor("gc", [P, E], F32, kind="ExternalInput")
    out_d = nc.dram_tensor("out", [L, P], F32, kind="ExternalOutput")

    with tile.TileContext(nc) as tc, ExitStack() as ctx:
        consts = ctx.enter_context(tc.tile_pool(name="consts", bufs=1))
        wpool = ctx.enter_context(tc.tile_pool(name="wpool", bufs=1))
        mixp = ctx.enter_context(tc.tile_pool(name="mixp", bufs=1))
        smallp = ctx.enter_context(tc.tile_pool(name="smallp", bufs=1))
        psy = ctx.enter_context(tc.tile_pool(name="psy", bufs=1, space="PSUM"))
        pss = ctx.enter_context(tc.tile_pool(name="pss", bufs=1, space="PSUM"))

        # Weight stream: ALL weights on the sync ring (the two HW-DGE
        # rings share one descriptor-processing pool and the scalar ring
        # only gets service after sync's queued work).  4 chunks, front-
        # loaded first chunk, small final chunk so the exposed completion
        # lag (sem16 = bytes + ~1.1us) covers only one k-pair.
        xw_t = wpool.tile([P, XWC], FP8)
        nc.sync.dma_start(out=xw_t[:, 0:XT8 + 3072],
                          in_=xw_d.ap()[:, 0:XT8 + 3072])
        # ep leads the scalar ring; gc's consumer (the gb build) is not
        # needed until the mix.
        ep_t = consts.tile([E, EPACK2], BF16)
        nc.scalar.dma_start(out=ep_t, in_=ep_d.ap())
        gc_t = consts.tile([P, E], F32)
        nc.scalar.dma_start(out=gc_t, in_=gc_d.ap())
        nc.sync.dma_start(out=xw_t[:, XT8 + 3072:XT8 + 5120],
                          in_=xw_d.ap()[:, XT8 + 3072:XT8 + 5120])
        nc.sync.dma_start(out=xw_t[:, XT8 + 5120:XT8 + 7168],
                          in_=xw_d.ap()[:, XT8 + 5120:XT8 + 7168])
        nc.sync.dma_start(out=xw_t[:, XT8 + 7168:XWC],
                          in_=xw_d.ap()[:, XT8 + 7168:XWC])
        xt_t = xw_t[:, 0:XT8].bitcast(BF16).rearrange(
            "p (k b) -> p k b", k=KT)
        w1_t = xw_t[:, XT8:XWC].rearrange("p (k h e c) -> p k h e c",
                                          k=KT, h=2, e=E)
        o = 0
        gt_t = ep_t[:, o:o + B]; o += B
        o += OC                                   # db (unused on-device)
        ow_t = ep_t[:, o:o + L * OC].rearrange(
            "e (h l c) -> e h l c", h=2, l=L); o += L * OC
        ob_t = ep_t[:, o:o + L]; o += L
        gtz_t = ep_t[:, o:o + P]; o += P         # gates.T | zeros
        dbr_t = ep_t[:, o:o + 2 * E * HC]        # partition-0 bias rows
        ones1_t = smallp.tile([1, B], BF16)
        nc.vector.memset(ones1_t[:], SW)         # carries the fp8 dequant

        # gate-broadcast table gb[p, (e, hc)] = g[b, e], built early on the
        # DVE (hidden under the w1 stream).
        ones_t = smallp.tile([P, HC], F32)
        nc.vector.memset(ones_t[:], 1.0)
        gb_t = consts.tile([P, E, HC], F32)
        for e in range(E):
            nc.vector.tensor_scalar_mul(gb_t[:, e, :], ones_t[:],
                                        gc_t[:, e:e + 1])
        # stage-2 accum staging [128, L] + identity for the PE transpose
        # that flips it to [L, 128] (so the output DMA is 2 descriptors,
        # not 128).  Identity builds early on the idle gpsimd engine.
        st2_t = smallp.tile([P, L], F32)
        ident_t = smallp.tile([P, P], F32)
        masks.make_identity(nc, ident_t[:])

        # ---- stage 1: y[64h+b, (e, hc)] = Sw * x . dense_w[e, oc_half, :]
        # h=0 / h=1 matmuls write PSUM partition halves 0-63 / 64-127 =
        # disjoint PE col-groups -> concurrent streams.  k-outer so the PE
        # consumes each chunk as it lands.  lhsT bf16 x rhs fp8e3.
        psum_y = psy.tile([P, E, HC], F32)

        # dense_b rides the PSUM group as a K=1 matmul (Sw-row x bias
        # row) and STARTS the accumulation group, so the group closes at
        # k7 exactly and the mix begins ~300ns sooner.  It only needs ep,
        # which the scalar ring lands well before chunk 0's semaphore.
        for h in range(2):
            nc.tensor.matmul(
                psum_y[h * 64:h * 64 + 64, :, :].rearrange("b e c -> b (e c)"),
                ones1_t[:],
                dbr_t[0:1, h * E * HC:(h + 1) * E * HC],
                start=True, stop=False, skip_group_check=True,
            )

        def big_matmul(k):
            for h in range(2):
                nc.tensor.matmul(
                    psum_y[h * 64:h * 64 + 64, :, :].rearrange(
                        "b e c -> b (e c)"),
                    xt_t[:, k, :],
                    w1_t[:, k, h].rearrange("p e c -> p (e c)"),
                    start=False,
                    stop=(k == KT - 1),
                    skip_group_check=True,
                )

        for k in range(KT):
            big_matmul(k)

        # ---- small matmuls after the k-loop ----
        # sel_ow^h [64h+b, (l, hc)] ; sel_ob [p, l]
        psum_ow = pss.tile([P, L, HC], F32)
        for h in range(2):
            sl = slice(h * 64, h * 64 + 64)
            nc.tensor.matmul(
                psum_ow[sl, :, :].rearrange("b l c -> b (l c)"),
                gt_t, ow_t[:, h].rearrange("e l c -> e (l c)"),
                start=True, stop=True, skip_group_check=True,
            )
        psum_ob = pss.tile([P, L], F32)
        nc.tensor.matmul(psum_ob[:], gtz_t, ob_t, start=True, stop=True)
        # Copies run on the scalar engine (ACT COPY), idle until tanh.
        # SBUF copy so stage 2 reads SBUF; col HC carries sel_ob so stage
        # 2's accumulate emits the final logits.
        sow_t = smallp.tile([P, L, HC + 1], BF16)
        nc.scalar.copy(sow_t[:, :, 0:HC], psum_ow[:])
        nc.scalar.copy(
            sow_t[:, :, HC:HC + 1].rearrange("p l c -> p (l c)"), psum_ob[:])

        # ---- gate mix: acc[p, hc] = sum_e gb[p,e,:] * psum_y[p,e,:] ----
        # contiguous pairwise tree over e; bf16 intermediates double the
        # DVE rate (a strided tensor_reduce over e measured slower).
        prod_t = mixp.tile([P, E, HC], BF16)
        nc.vector.tensor_tensor(
            out=prod_t[:], in0=psum_y[:], in1=gb_t[:], op=OP.mult,
        )
        t1 = mixp.tile([P, 4, HC], BF16)
        nc.vector.tensor_add(t1[:], prod_t[:, 0:4, :], prod_t[:, 4:8, :])
        t2 = mixp.tile([P, 2, HC], BF16)
        nc.vector.tensor_add(t2[:], t1[:, 0:2, :], t1[:, 2:4, :])
        acc_t = mixp.tile([P, HC], BF16)
        nc.vector.tensor_add(acc_t[:], t2[:, 0, :], t2[:, 1, :])

        # tanh (scale folds the 1/Sw dequant) into cols 0:HC of t_ext;
        # col HC holds 1.0 so stage 2's accumulate picks up sel_ob.
        t_ext = smallp.tile([P, HC + 1], BF16)
        nc.gpsimd.memset(t_ext[:, HC:HC + 1], 1.0)
        nc.scalar.activation(t_ext[:, 0:HC], acc_t[:], AF.Tanh,
                             scale=1.0 / SW)

        # ---- stage 2: st2[64h+b, l] = sum_hc t * sel_ow (+ sel_ob) ----
        # accum_out (free-dim sum) on the DVE, one op per label column;
        # then a PE transpose (long idle by now) flips [128,L] -> [L,128]
        # in PSUM and a scalar copy stages it in SBUF for the output DMA.
        dumpv = smallp.tile([P, HC + 1], BF16)
        for l in range(L):
            nc.vector.scalar_tensor_tensor(
                out=dumpv[:], in0=sow_t[:, l, :], scalar=1.0, in1=t_ext[:],
                op0=OP.mult, op1=OP.mult,
                accum_out=st2_t[:, l:l + 1])
        psum_tr = pss.tile([L, P], F32)
        nc.tensor.transpose(psum_tr[:], st2_t[:], ident_t[:])
        out_t = smallp.tile([L, P], F32)
        nc.scalar.copy(out_t[:], psum_tr[:])
        # Output DMA inside the tile context (ordered right after the
        # scalar copy, skipping the ~0.7us tile-end barrier an outside-
        # context dispatch pays) and on the SCALAR ring: the sync ring's
        # post-stream drain was observed to stall ~2.5us before joining
        # the epilogue rendezvous, and queueing the output there re-arms
        # that wait.  Only 2 descriptors ([L,128] staging).
        nc.scalar.dma_start(out=out_d.ap(), in_=out_t[:],
                            single_packet=True)

    nc.compile()
    return nc


def _prep_inputs(X, gates, dense_w, dense_b, out_w, out_b):
    """Host-side layout prep (slice/transpose/cast only) -> per-core maps."""
    X = np.asarray(X, dtype=np.float32)
    gates = np.asarray(gates, dtype=np.float32)
    dense_w = np.asarray(dense_w, dtype=np.float32)
    dense_b = np.asarray(dense_b, dtype=np.float32)
    out_w = np.asarray(out_w, dtype=np.float32)
    out_b = np.asarray(out_b, dtype=np.float32)

    xcls = X[:, 0, :]                                     # [B, H]
    # xt[i_lo, k, b] = x[b, k*128 + i_lo]
    xt = (xcls.T.reshape(KT, P, B).transpose(1, 0, 2)
          .reshape(P, XTC).astype(bfloat16))
    xt8 = xt.view(np.uint8)                               # [P, XT8]
    gt = gates.T                                          # [E, B]
    gtz = np.concatenate([gt, np.zeros_like(gt)], axis=1)  # [E, 128]
    gc2 = np.ascontiguousarray(np.vstack([gates, gates]))  # [128, E] fp32

    in_maps = []
    for c in range(NCORES):
        sl = slice(c * OC, (c + 1) * OC)
        # w1[i_lo, k, h, e, hc] = dense_w[e, c*OC + h*64 + hc, k*128 + i_lo]
        w1 = (dense_w[:, sl, :]                 # [E, OC, H]
              .reshape(E, 2, HC, KT, P)         # [e, h, hc, k, i_lo]
              .transpose(4, 3, 1, 0, 2)         # [i_lo, k, h, e, hc]
              .reshape(P, W1C))
        w18 = np.clip(w1 * np.float32(SW), -15.5, 15.5).astype(float8_e3m4)
        xw = np.concatenate(
            [xt8, w18.view(np.uint8)], axis=1).view(float8_e3m4)

        # ow2[e, (h, l, hc)] = out_w[e, l, c*OC + h*64 + hc]
        ow2 = (out_w[:, :, sl].reshape(E, L, 2, HC)
               .transpose(0, 2, 1, 3).reshape(E, L * OC))
        ob = out_b if c == 0 else np.zeros_like(out_b)
        # dbrow[h, (e, hc)] = dense_b[e, c*OC + h*64 + hc], in E-partition
        # rows 0/1 (rows 2..E-1 zero) for the K=1 bias matmul.
        dbrow = np.zeros((E, 2 * E * HC), dtype=np.float32)
        dbrow[0] = (dense_b[:, sl].reshape(E, 2, HC)
                    .transpose(1, 0, 2).reshape(2 * E * HC))
        ep = np.concatenate(
            [gt, dense_b[:, sl], ow2, ob, gtz, dbrow],
            axis=1).astype(bfloat16)
        in_maps.append({
            "xw": np.ascontiguousarray(xw),
            "ep": np.ascontiguousarray(ep),
            "gc": gc2,
        })
    return in_maps


def _run(in_maps, trace=False, tmpdir=None):
    global _cached
    from concourse.bass_utils import run_bass_kernel_spmd

    if _cached is None:
        _cached = _build()
    res = run_bass_kernel_spmd(
        _cached, in_maps, list(range(NCORES)), trace=trace, tmpdir=tmpdir,
    )
    return res


def kernel(X, gates, dense_w, dense_b, out_w, out_b):
    in_maps = _prep_inputs(X, gates, dense_w, dense_b, out_w, out_b)
    res = _run(in_maps)
    acc = np.zeros((B, L), dtype=np.float64)
    for c in range(NCORES):
        part = res.results[c]["out"].astype(np.float64)   # [L, 128]
        acc += part.reshape(L, 2, B).sum(axis=1).T
    return acc.astype(np.float32)


# revision 11
# speedup vs baseline: 1.6467x; 1.6467x over previous
"""Trainium2 Bass kernel for nn_MoEsparseRoutingForClassification.

Reference computation (B=64, S=128, H=1024, E=8, L=2):
    x = X[:, 0, :]                                   # CLS token [B,H]
    y[b,o]   = sum_e g[b,e] * (x[b] . dense_w[e,o,:]) + (g @ dense_b)[b,o]
    t        = tanh(y)
    out[b,l] = sum_e g[b,e] * (t[b] . out_w[e,l,:])  + (g @ out_b)[b,l]

Distribution: the H output dim of the dense layer is sharded 8 ways
(OC=128 per core).  Core c computes y[:, c*OC:(c+1)*OC], applies tanh,
and contracts against out_w[:, :, c_slice] for a partial [B,L] logit;
partials sum on the host.  No cross-core collective.

v2 (fp8 stream): dense_w streams as float8 e3m4 (4 mantissa bits),
pre-scaled by Sw=128 so the N(0,0.02) weights land in e3m4's normal
range (max 15.5); the CLS block stays bf16 (its bytes ride the same
fp8 DRAM tensor via bitcast) and the PE runs mixed bf16(stationary)
x fp8(moving) matmuls.  HBM traffic drops to ~1.2 MiB/core.  The
Sw dequant is folded into existing ops: the dense_b K=1 ride-along
matmul uses a ones-row of value Sw (so PSUM holds Sw*(x.W + db))
and the tanh activation applies scale=1/Sw.  Host-measured rel err
~1.4e-2 (tolerance 2e-2); bf16 everywhere was 5.3e-3.

Scheduling (from NTFF traces of the bf16 predecessor @25.9us):
  - ~6.3us framework preamble before the first DMA dispatch and ~9us
    epilogue cascade after the last instruction are fixed costs, but
    the measured exec window appears to START after the preamble while
    INCLUDING the epilogue - so tail latency matters more than head.
  - weight stream all on the sync HW-DGE ring, 4 chunks the PE chases
    (xt+k0k1k2 | k3k4 | k5k6 | k7); ep+gc on the scalar ring which
    only gets descriptor service after sync's queued work - with the
    2x shorter fp8 stream the scalar-ring ep lands AFTER the PE's
    k1->k2 bubble, so the dense_b ride-along matmul moved to after k7
    (the PE queue is in-order; a parked not-ready matmul stalls it).
  - gate mix: tensor_tensor mult then a single strided tensor_reduce
    over e (fp32 acc out), replacing the 3-add pairwise tree.
  - output staging transposed on the DVE ([128,2] -> [2,128] via
    32x32 stream-transpose blocks) so the final DMA is 8 descriptors
    instead of 128: the teardown's wait on descriptor-completion
    pacing (~25ns each) was ~3us of the epilogue.
"""

import sys

import numpy as np

for _p in ("/opt/trn_rl_repo",):
    if _p not in sys.path:
        sys.path.insert(0, _p)

# If the environment sets BASS_TRACE but lacks antenv.axon_hooks (this agent
# image does), run_bass_kernel_spmd would crash on import; pre-seed a no-op
# module so tracing degrades gracefully instead.
try:  # pragma: no cover
    import antenv.axon_hooks  # noqa: F401
except Exception:  # pragma: no cover
    import types as _types

    _m = _types.ModuleType("antenv.axon_hooks")
    _m._hook = None
    _m.set_axon_ntff_profile_hook = lambda h: setattr(_m, "_hook", h)
    _m.get_axon_ntff_profile_hook = lambda: _m._hook
    sys.modules["antenv.axon_hooks"] = _m

from ml_dtypes import bfloat16, float8_e3m4  # noqa: E402

B, S, H = 64, 128, 1024
E, L = 8, 2
NCORES = 8
OC = H // NCORES          # dense-output slice per core (128)
HC = OC // 2              # half-slice mapped to a PSUM partition half (64)
KT = H // 128             # contraction tiles
P = 128

SW = 128.0                # fp8 pre-scale for dense_w (power of 2, exact)

XTC = KT * B              # xt cols (bf16 units) in the packed stream (512)
XT8 = 2 * XTC             # same bytes counted in fp8 cols (1024)
W1C = KT * 2 * E * HC     # w1 cols (8192, fp8)
XWC = XT8 + W1C           # packed stream width in fp8 cols (9216)

_cached = None


def _build():
    from contextlib import ExitStack

    import concourse.tile as tile
    from concourse import bacc, masks, mybir

    F32 = mybir.dt.float32
    BF16 = mybir.dt.bfloat16
    FP8 = mybir.dt.float8e3
    AF = mybir.ActivationFunctionType
    OP = mybir.AluOpType
    AX = mybir.AxisListType

    nc = bacc.Bacc("TRN2", target_bir_lowering=False, debug=False,
                   num_devices=NCORES)

    # E-pack along the free dim (one DMA for all E-partition consts):
    #   gt [E,B] | db [E,2,HC] | ow2 [E,2,L,HC] | ob [E,L] | gtz [E,P]
    EPACK = B + OC + L * OC + L + P      # 64+128+256+2+128 = 578
    # + a 1024-col dense-bias row block in PARTITION 0 ONLY (matmul
    # operands must sit at base partition 0/32/64): cols h*512+(e,hc) =
    # dense_b[e, h*64+hc], for the K=1 bias matmul
    # + an [E, E*HC] 0/1 indicator block: gb[b, (e,c)] = g[b,e] builds on
    # the PE as gt.T @ ind (kills the old [128,E] gc DMA whose 128 tiny
    # descriptors backlogged the completion-ack pipe).
    EPACK2 = EPACK + 2 * E * HC          # 578+1024
    EPACK3 = EPACK2 + E * HC             # +512
    xw_d = nc.dram_tensor("xw", [P, XWC], FP8, kind="ExternalInput")
    ep_d = nc.dram_tensor("ep", [E, EPACK3], BF16, kind="ExternalInput")
    out_d = nc.dram_tensor("out", [L, P], F32, kind="ExternalOutput")

    with tile.TileContext(nc) as tc, ExitStack() as ctx:
        consts = ctx.enter_context(tc.tile_pool(name="consts", bufs=1))
        wpool = ctx.enter_context(tc.tile_pool(name="wpool", bufs=1))
        mixp = ctx.enter_context(tc.tile_pool(name="mixp", bufs=1))
        smallp = ctx.enter_context(tc.tile_pool(name="smallp", bufs=1))
        psy = ctx.enter_context(tc.tile_pool(name="psy", bufs=1, space="PSUM"))
        pss = ctx.enter_context(tc.tile_pool(name="pss", bufs=1, space="PSUM"))

        # Weight stream: ALL weights on the sync ring (the two HW-DGE
        # rings share one descriptor-processing pool and the scalar ring
        # only gets service after sync's queued work).  4 chunks, front-
        # loaded first chunk, small final chunk so the exposed completion
        # lag (sem16 = bytes + ~1.1us) covers only one k-pair.
        xw_t = wpool.tile([P, XWC], FP8)
        nc.sync.dma_start(out=xw_t[:, 0:XT8 + 3072],
                          in_=xw_d.ap()[:, 0:XT8 + 3072])
        # ep leads the scalar ring; gc's consumer (the gb build) is not
        # needed until the mix.
        ep_t = consts.tile([E, EPACK2], BF16)
        nc.scalar.dma_start(out=ep_t, in_=ep_d.ap())
        gc_t = consts.tile([P, E], F32)
        nc.scalar.dma_start(out=gc_t, in_=gc_d.ap())
        nc.sync.dma_start(out=xw_t[:, XT8 + 3072:XT8 + 5120],
                          in_=xw_d.ap()[:, XT8 + 3072:XT8 + 5120])
        nc.sync.dma_start(out=xw_t[:, XT8 + 5120:XT8 + 7168],
                          in_=xw_d.ap()[:, XT8 + 5120:XT8 + 7168])
        nc.sync.dma_start(out=xw_t[:, XT8 + 7168:XWC],
                          in_=xw_d.ap()[:, XT8 + 7168:XWC])
        xt_t = xw_t[:, 0:XT8].bitcast(BF16).rearrange(
            "p (k b) -> p k b", k=KT)
        w1_t = xw_t[:, XT8:XWC].rearrange("p (k h e c) -> p k h e c",
                                          k=KT, h=2, e=E)
        o = 0
        gt_t = ep_t[:, o:o + B]; o += B
        o += OC                                   # db (unused on-device)
        ow_t = ep_t[:, o:o + L * OC].rearrange(
            "e (h l c) -> e h l c", h=2, l=L); o += L * OC
        ob_t = ep_t[:, o:o + L]; o += L
        gtz_t = ep_t[:, o:o + P]; o += P         # gates.T | zeros
        dbr_t = ep_t[:, o:o + 2 * E * HC]        # partition-0 bias rows
        ones1_t = smallp.tile([1, B], BF16)
        nc.vector.memset(ones1_t[:], SW)         # carries the fp8 dequant

        # gate-broadcast table gb[p, (e, hc)] = g[b, e], built early on the
        # DVE (hidden under the w1 stream).
        ones_t = smallp.tile([P, HC], F32)
        nc.vector.memset(ones_t[:], 1.0)
        gb_t = consts.tile([P, E, HC], F32)
        for e in range(E):
            nc.vector.tensor_scalar_mul(gb_t[:, e, :], ones_t[:],
                                        gc_t[:, e:e + 1])
        # stage-2 accum staging [128, L] + identity for the PE transpose
        # that flips it to [L, 128] (so the output DMA is 2 descriptors,
        # not 128).  Identity builds early on the idle gpsimd engine.
        st2_t = smallp.tile([P, L], F32)
        ident_t = smallp.tile([P, P], F32)
        masks.make_identity(nc, ident_t[:])

        # ---- stage 1: y[64h+b, (e, hc)] = Sw * x . dense_w[e, oc_half, :]
        # h=0 / h=1 matmuls write PSUM partition halves 0-63 / 64-127 =
        # disjoint PE col-groups -> concurrent streams.  k-outer so the PE
        # consumes each chunk as it lands.  lhsT bf16 x rhs fp8e3.
        psum_y = psy.tile([P, E, HC], F32)

        # dense_b rides the PSUM group as a K=1 matmul (Sw-row x bias
        # row) and STARTS the accumulation group, so the group closes at
        # k7 exactly and the mix begins ~300ns sooner.  It only needs ep,
        # which the scalar ring lands well before chunk 0's semaphore.
        for h in range(2):
            nc.tensor.matmul(
                psum_y[h * 64:h * 64 + 64, :, :].rearrange("b e c -> b (e c)"),
                ones1_t[:],
                dbr_t[0:1, h * E * HC:(h + 1) * E * HC],
                start=True, stop=False, skip_group_check=True,
            )

        def big_matmul(k):
            for h in range(2):
                nc.tensor.matmul(
                    psum_y[h * 64:h * 64 + 64, :, :].rearrange(
                        "b e c -> b (e c)"),
                    xt_t[:, k, :],
                    w1_t[:, k, h].rearrange("p e c -> p (e c)"),
                    start=False,
                    stop=(k == KT - 1),
                    skip_group_check=True,
                )

        for k in range(KT):
            big_matmul(k)

        # ---- small matmuls after the k-loop ----
        # sel_ow^h [64h+b, (l, hc)] ; sel_ob [p, l]
        psum_ow = pss.tile([P, L, HC], F32)
        for h in range(2):
            sl = slice(h * 64, h * 64 + 64)
            nc.tensor.matmul(
                psum_ow[sl, :, :].rearrange("b l c -> b (l c)"),
                gt_t, ow_t[:, h].rearrange("e l c -> e (l c)"),
                start=True, stop=True, skip_group_check=True,
            )
        psum_ob = pss.tile([P, L], F32)
        nc.tensor.matmul(psum_ob[:], gtz_t, ob_t, start=True, stop=True)
        # Copies run on the scalar engine (ACT COPY), idle until tanh.
        # SBUF copy so stage 2 reads SBUF; col HC carries sel_ob so stage
        # 2's accumulate emits the final logits.
        sow_t = smallp.tile([P, L, HC + 1], BF16)
        nc.scalar.copy(sow_t[:, :, 0:HC], psum_ow[:])
        nc.scalar.copy(
            sow_t[:, :, HC:HC + 1].rearrange("p l c -> p (l c)"), psum_ob[:])

        # ---- gate mix: acc[p, hc] = sum_e gb[p,e,:] * psum_y[p,e,:] ----
        # contiguous pairwise tree over e; bf16 intermediates double the
        # DVE rate (a strided tensor_reduce over e measured slower).
        prod_t = mixp.tile([P, E, HC], BF16)
        nc.vector.tensor_tensor(
            out=prod_t[:], in0=psum_y[:], in1=gb_t[:], op=OP.mult,
        )
        t1 = mixp.tile([P, 4, HC], BF16)
        nc.vector.tensor_add(t1[:], prod_t[:, 0:4, :], prod_t[:, 4:8, :])
        t2 = mixp.tile([P, 2, HC], BF16)
        nc.vector.tensor_add(t2[:], t1[:, 0:2, :], t1[:, 2:4, :])
        acc_t = mixp.tile([P, HC], BF16)
        nc.vector.tensor_add(acc_t[:], t2[:, 0, :], t2[:, 1, :])

        # tanh (scale folds the 1/Sw dequant) into cols 0:HC of t_ext;
        # col HC holds 1.0 so stage 2's accumulate picks up sel_ob.
        t_ext = smallp.tile([P, HC + 1], BF16)
        nc.gpsimd.memset(t_ext[:, HC:HC + 1], 1.0)
        nc.scalar.activation(t_ext[:, 0:HC], acc_t[:], AF.Tanh,
                             scale=1.0 / SW)

        # ---- stage 2: st2[64h+b, l] = sum_hc t * sel_ow (+ sel_ob) ----
        # accum_out (free-dim sum) on the DVE, one op per label column;
        # then a PE transpose (long idle by now) flips [128,L] -> [L,128]
        # in PSUM and a scalar copy stages it in SBUF for the output DMA.
        dumpv = smallp.tile([P, HC + 1], BF16)
        for l in range(L):
            nc.vector.scalar_tensor_tensor(
                out=dumpv[:], in0=sow_t[:, l, :], scalar=1.0, in1=t_ext[:],
                op0=OP.mult, op1=OP.mult,
                accum_out=st2_t[:, l:l + 1])
        psum_tr = pss.tile([L, P], F32)
        nc.tensor.transpose(psum_tr[:], st2_t[:], ident_t[:])
        out_t = smallp.tile([L, P], F32)
        nc.scalar.copy(out_t[:], psum_tr[:])
        # Output DMA inside the tile context (ordered right after the
        # scalar copy, skipping the ~0.7us tile-end barrier an outside-
        # context dispatch pays) and on the SCALAR ring: the sync ring's
        # post-stream drain was observed to stall ~2.5us before joining
        # the epilogue rendezvous, and queueing the output there re-arms
        # that wait.  Only 2 descriptors ([L,128] staging).
        nc.scalar.dma_start(out=out_d.ap(), in_=out_t[:],
                            single_packet=True)

    nc.compile()
    return nc


def _prep_inputs(X, gates, dense_w, dense_b, out_w, out_b):
    """Host-side layout prep (slice/transpose/cast only) -> per-core maps."""
    X = np.asarray(X, dtype=np.float32)
    gates = np.asarray(gates, dtype=np.float32)
    dense_w = np.asarray(dense_w, dtype=np.float32)
    dense_b = np.asarray(dense_b, dtype=np.float32)
    out_w = np.asarray(out_w, dtype=np.float32)
    out_b = np.asarray(out_b, dtype=np.float32)

    xcls = X[:, 0, :]                                     # [B, H]
    # xt[i_lo, k, b] = x[b, k*128 + i_lo]
    xt = (xcls.T.reshape(KT, P, B).transpose(1, 0, 2)
          .reshape(P, XTC).astype(bfloat16))
    xt8 = xt.view(np.uint8)                               # [P, XT8]
    gt = gates.T                                          # [E, B]
    gtz = np.concatenate([gt, np.zeros_like(gt)], axis=1)  # [E, 128]
    gc2 = np.ascontiguousarray(np.vstack([gates, gates]))  # [128, E] fp32

    in_maps = []
    for c in range(NCORES):
        sl = slice(c * OC, (c + 1) * OC)
        # w1[i_lo, k, h, e, hc] = dense_w[e, c*OC + h*64 + hc, k*128 + i_lo]
        w1 = (dense_w[:, sl, :]                 # [E, OC, H]
              .reshape(E, 2, HC, KT, P)         # [e, h, hc, k, i_lo]
              .transpose(4, 3, 1, 0, 2)         # [i_lo, k, h, e, hc]
              .reshape(P, W1C))
        w18 = np.clip(w1 * np.float32(SW), -15.5, 15.5).astype(float8_e3m4)
        xw = np.concatenate(
            [xt8, w18.view(np.uint8)], axis=1).view(float8_e3m4)

        # ow2[e, (h, l, hc)] = out_w[e, l, c*OC + h*64 + hc]
        ow2 = (out_w[:, :, sl].reshape(E, L, 2, HC)
               .transpose(0, 2, 1, 3).reshape(E, L * OC))
        ob = out_b if c == 0 else np.zeros_like(out_b)
        # dbrow[h, (e, hc)] = dense_b[e, c*OC + h*64 + hc], in E-partition
        # rows 0/1 (rows 2..E-1 zero) for the K=1 bias matmul.
        dbrow = np.zeros((E, 2 * E * HC), dtype=np.float32)
        dbrow[0] = (dense_b[:, sl].reshape(E, 2, HC)
                    .transpose(1, 0, 2).reshape(2 * E * HC))
        ep = np.concatenate(
            [gt, dense_b[:, sl], ow2, ob, gtz, dbrow],
            axis=1).astype(bfloat16)
        in_maps.append({
            "xw": np.ascontiguousarray(xw),
            "ep": np.ascontiguousarray(ep),
            "gc": gc2,
        })
    return in_maps


def _run(in_maps, trace=False, tmpdir=None):
    global _cached
    from concourse.bass_utils import run_bass_kernel_spmd

    if _cached is None:
        _cached = _build()
    res = run_bass_kernel_spmd(
        _cached, in_maps, list(range(NCORES)), trace=trace, tmpdir=tmpdir,
    )
    return res


def kernel(X, gates, dense_w, dense_b, out_w, out_b):
    in_maps = _prep_inputs(X, gates, dense_w, dense_b, out_w, out_b)
    res = _run(in_maps)
    acc = np.zeros((B, L), dtype=np.float64)
    for c in range(NCORES):
        part = res.results[c]["out"].astype(np.float64)   # [L, 128]
        acc += part.reshape(L, 2, B).sum(axis=1).T
    return acc.astype(np.float32)
